# revision 1
# baseline (speedup 1.0000x reference)
"""CNF vector-field + exact Jacobian-trace kernel for Trainium2 (8 NeuronCores).

Math: for each sample x (D=32), with inp = [x, t] (33,):
  h1 = tanh(inp @ W1 + b1); h2 = tanh(h1 @ W2 + b2); dx = h2 @ W3 + b3
  div = trace(J),  J = W1r D1 W2 D2 W3  (D_i = diag(1 - h_i^2), W1r = W1[:32])
      = d1^T C d2,  C = W2 * (W3 @ W1r)^T   (elementwise *)
  out = [dx, div]  (B, 33)

Implementation notes:
  - data-parallel over batch (2048 -> 8 x 256), weights replicated
  - feature-major on-device layout: weights are natural pre-transposed lhsT
  - P = -C;  gt = P^T h1sq - (P^T 1);  E = (h2sq - 1) * gt = gt * d2 * (-1)
    div = (-1)^T E  -- the "1 - x^2" affines fold into matmuls / fused DVE ops
  - matmuls run as float32r (TF32-like, 4x faster than fp32 at N>=256)
  - consolidated DMAs via 3-D access patterns; W2 (the big one) issued last
  - engine streams are in-order: emission order is tuned so PE/ACT/DVE/Pool
    overlap (P-chain early, vp after z2, copies on ACT, h2sq on GpSimd)
"""
import sys

for _p in ("/opt/trn_rl_repo", "/root/.axon_site/_ro/trn_rl_repo"):
    if _p not in sys.path:
        sys.path.append(_p)

import numpy as np

B, D, H = 2048, 32, 512
NCORES = 8
BC = B // NCORES          # 256 rows per core
NK = H // 128             # 4 chunks of the hidden dim

_CACHE = {}


def _build(reps=None):
    import contextlib
    import concourse.bass as bass
    import concourse.tile as tile
    from concourse import bacc, mybir
    from concourse.masks import make_identity

    f32 = mybir.dt.float32
    f32r = mybir.dt.float32r
    AF = mybir.ActivationFunctionType
    ALU = mybir.AluOpType

    nc = bacc.Bacc("TRN2", target_bir_lowering=False, debug=False,
                   num_devices=NCORES)

    x_ext = nc.dram_tensor("x", [BC, D + 1], f32, kind="ExternalInput").ap()
    # w1 = [W1; b1] stacked then column-interleaved on host -> (16, 34, 32):
    # w1i[a, r, b] = w1s[r, a*32 + b]. The interleave makes the DMA split
    # into 34*16 non-contiguous descriptors so all 16 HWDGE queues are busy
    # (DMAs that leave queues empty get ~4us-late completion semaphores).
    w1_ext = nc.dram_tensor("w1", [16, D + 2, 32], f32r, kind="ExternalInput").ap()
    w2_ext = nc.dram_tensor("w2", [H, H], f32r, kind="ExternalInput").ap()
    w3_ext = nc.dram_tensor("w3", [H, D], f32r, kind="ExternalInput").ap()
    # colpack cols: 0=+1, 1=-1, 2=unused, 3:7=b2 column-major,
    # 7:11 = bias1 = t*W1[32,:]+b1 column-major (host-derived weight constant)
    colp_ext = nc.dram_tensor("colp", [128, 11], f32r, kind="ExternalInput").ap()
    # rowpack: [0:256]=ones, [256:288]=b3; host-padded to (16, 32) rows with
    # data in cols 0:18 so the DMA emits 16 strided descriptors (see w1 note)
    rowp_ext = nc.dram_tensor("rowp", [16, 32], f32r, kind="ExternalInput").ap()
    out_ext = nc.dram_tensor("out", [BC, D + 1], f32, kind="ExternalOutput").ap()

    with tile.TileContext(nc) as tc:
        with tc.tile_pool(name="const", bufs=1) as cpool, \
             tc.tile_pool(name="work", bufs=1) as wpool, \
             tc.tile_pool(name="ps", bufs=1, space="PSUM") as pps, \
             (tc.For_i(0, reps, 1) if reps else contextlib.nullcontext()):

            def big_ps(nm):
                return pps.tile([128, H], f32, name=nm, tag="big", bufs=6)

            def small_ps(nm, shape):
                return pps.tile(shape, f32, name=nm, tag="small", bufs=2)

            # -------- ACT spline-table preload (overlaps the DMA phase) -----
            dm0 = wpool.tile([1, 1], f32, name="dm0")
            dm1 = wpool.tile([1, 1], f32, name="dm1")
            nc.gpsimd.memset(dm0[:, :], 0.0)
            nc.scalar.activation(dm1[:, :], dm0[:, :], AF.Tanh)

            # ------------- input DMAs (few, large; W2 last) -------------
            w1e = cpool.tile([D + 2, H], f32r, name="w1e")   # 0:33 = W1, 33 = b1
            nc.sync.dma_start(
                out=w1e[:, :].rearrange("r (a b) -> r a b", a=16),
                in_=w1_ext.rearrange("a r b -> r a b"))

            colp = cpool.tile([128, 11], f32r, name="colp")
            nc.sync.dma_start(out=colp[:, :], in_=colp_ext[:, :])
            ones_col = colp[:, 0:1]
            neg_col = colp[:, 1:2]

            w3all = cpool.tile([128, NK * D], f32r, name="w3all")
            nc.sync.dma_start(
                out=w3all[:, :].rearrange("p (k j) -> p k j", k=NK),
                in_=w3_ext.rearrange("(k p) j -> p k j", k=NK))
            w3k = [w3all[:, k * D:(k + 1) * D] for k in range(NK)]

            xall = wpool.tile([128, 2 * (D + 1)], f32, name="xall")
            nc.scalar.dma_start(
                out=xall[:, :].rearrange("p (i c) -> p i c", i=2),
                in_=x_ext.rearrange("(i p) c -> p i c", i=2))

            w2all = cpool.tile([128, NK * H], f32r, name="w2all")
            nc.sync.dma_start(
                out=w2all[:, :].rearrange("p (k j) -> p k j", k=NK),
                in_=w2_ext.rearrange("(k p) j -> p k j", k=NK))
            w2k = [w2all[:, k * H:(k + 1) * H] for k in range(NK)]

            rowp = cpool.tile([1, BC + D], f32r, name="rowp")
            nc.sync.dma_start(
                out=rowp[:, :].rearrange("p (a b) -> p a b", a=16),
                in_=rowp_ext[:, 0:18].rearrange("(o a) b -> o a b", o=1))
            ones_row = rowp[:, 0:BC]
            b3row = rowp[:, BC:BC + D]

            ident = cpool.tile([128, 128], f32, name="ident")
            make_identity(nc, ident[:, :])

            # ------- W3^T (negated): PE transposes + DVE negate-copies -------
            negw3t = wpool.tile([D, H], f32r, name="negw3t")
            for k in range(NK):
                wp = small_ps("w3tp", [D, 128])
                nc.tensor.transpose(wp[:, :], w3k[k].bitcast(f32), ident[:, :])
                nc.vector.tensor_scalar(out=negw3t[:, k * 128:(k + 1) * 128],
                                        in0=wp[:, :], scalar1=-1.0, scalar2=None,
                                        op0=ALU.mult)

            # ---------------- x transpose: A0 = xs^T (32, 256) ----------------
            a0 = wpool.tile([D, BC], f32r, name="a0")
            for i in range(2):
                xp = small_ps("xT", [D + 1, 128])
                nc.tensor.transpose(xp[:, :], xall[:, i * (D + 1):(i + 1) * (D + 1)],
                                    ident[:, :])
                nc.vector.tensor_copy(a0[:, i * 128:(i + 1) * 128], xp[0:D, :])

            # ---------------- layer 1 matmuls, then all tanh ----------------
            z1s = []
            for m in range(NK):
                z1 = big_ps("z1")
                nc.tensor.matmul(z1[:, 0:BC], w1e[0:D, m * 128:(m + 1) * 128],
                                 a0[:, :], start=True, stop=True)
                z1s.append(z1)
            h1t = []
            for m in range(NK):
                h = wpool.tile([128, BC], f32r, name=f"h1t_{m}")
                nc.scalar.activation(h[:, :], z1s[m][:, 0:BC], AF.Tanh,
                                     bias=colp[:, 7 + m:8 + m].bitcast(f32))
                h1t.append(h)

            # ---------------- P = -(W2 * M^T), M = W3 @ W1r ----------------
            pmat = []
            for m in range(NK):
                mp = big_ps("negMt")
                nc.tensor.matmul(mp[:, :], w1e[0:D, m * 128:(m + 1) * 128],
                                 negw3t[:, :], start=True, stop=True)
                p = cpool.tile([128, H], f32r, name=f"p_{m}")
                nc.vector.tensor_tensor(out=p[:, :], in0=w2k[m].bitcast(f32),
                                        in1=mp[:, :], op=ALU.mult)
                pmat.append(p)

            # ---------------- vP row (early: gates the div tail) ------------
            vp_ps = small_ps("vp_ps", [1, H])
            for k in range(NK):
                nc.tensor.matmul(vp_ps[:, :], ones_col, pmat[k][:, :],
                                 start=(k == 0), stop=(k == NK - 1))
            vneg = wpool.tile([1, H], f32r, name="vneg")
            nc.scalar.activation(vneg[:, :], vp_ps[:, :], AF.Copy, scale=-1.0)

            # ---------------- h1sq on DVE (f32r, feeds gt matmuls) ----------
            h1sq = []
            for m in range(NK):
                sq = wpool.tile([128, BC], f32r, name=f"h1sq_{m}")
                nc.vector.tensor_tensor(out=sq[:, :], in0=h1t[m][:, :].bitcast(f32),
                                        in1=h1t[m][:, :].bitcast(f32), op=ALU.mult)
                h1sq.append(sq)

            # ---------------- layer 2 ----------------
            # k-outer so each z2[m] consumes h1t[k] as soon as tanh1[k] lands
            z2s = [big_ps("z2") for _ in range(NK)]
            for k in range(NK):
                for m in range(NK):
                    nc.tensor.matmul(z2s[m][:, 0:BC],
                                     w2k[k][:, m * 128:(m + 1) * 128],
                                     h1t[k][:, :],
                                     start=(k == 0), stop=(k == NK - 1))
            h2t = []
            for m in range(NK):
                h = wpool.tile([128, BC], f32r, name=f"h2t_{m}")
                nc.scalar.activation(h[:, :], z2s[m][:, 0:BC], AF.Tanh,
                                     bias=colp[:, 3 + m:4 + m].bitcast(f32))
                h2t.append(h)

            # ---------------- h2sq on GpSimd (SBUF only) ----------------
            h2sq = []
            for m in range(NK):
                sq = wpool.tile([128, BC], f32, name=f"h2sq_{m}")
                nc.gpsimd.tensor_tensor(out=sq[:, :], in0=h2t[m][:, :].bitcast(f32),
                                        in1=h2t[m][:, :].bitcast(f32), op=ALU.mult)
                h2sq.append(sq)

            # ------- gt = P^T h1sq - vP ; E = (h2sq - 1) * gt = -gt*d2 -------
            # k-outer gt accumulation, same early-consume pipelining
            gts = [big_ps("gt") for _ in range(NK)]
            for k in range(NK):
                for m in range(NK):
                    nc.tensor.matmul(gts[m][:, 0:BC],
                                     pmat[k][:, m * 128:(m + 1) * 128],
                                     h1sq[k][:, :],
                                     start=(k == 0), stop=False)
            ee = []
            for m in range(NK):
                nc.tensor.matmul(gts[m][:, 0:BC], vneg[:, m * 128:(m + 1) * 128],
                                 ones_row, start=False, stop=True)
                e = wpool.tile([128, BC], f32r, name=f"e_{m}")
                nc.vector.scalar_tensor_tensor(out=e[:, :], in0=h2sq[m][:, :],
                                               scalar=1.0, in1=gts[m][:, 0:BC],
                                               op0=ALU.subtract, op1=ALU.mult)
                ee.append(e)

            # -------- dx = W3^T h2 + b3 ; div = (-1)^T E --------
            dx_ps = small_ps("dx_ps", [D, BC])
            for k in range(NK):
                nc.tensor.matmul(dx_ps[:, :], w3k[k], h2t[k][:, :],
                                 start=(k == 0), stop=False)
            nc.tensor.matmul(dx_ps[:, :], b3row, ones_row,
                             start=False, stop=True)
            outt = wpool.tile([D + 1, BC], f32, name="outt")
            nc.scalar.activation(outt[0:D, :], dx_ps[:, :], AF.Copy)
            div_ps = small_ps("div_ps", [1, BC])
            for k in range(NK):
                nc.tensor.matmul(div_ps[:, :], neg_col, ee[k][:, :],
                                 start=(k == 0), stop=(k == NK - 1))
            nc.scalar.activation(outt[D:D + 1, :], div_ps[:, :], AF.Copy)

            # ------- transpose back to (256, 33) and store -------
            outs = wpool.tile([128, 2 * (D + 1)], f32, name="outs")
            for i in range(2):
                op = small_ps("outP", [128, D + 1])
                nc.tensor.transpose(op[:, :], outt[:, i * 128:(i + 1) * 128],
                                    ident[0:D + 1, 0:D + 1])
                nc.scalar.activation(outs[:, i * (D + 1):(i + 1) * (D + 1)],
                                     op[:, :], AF.Copy)
            nc.scalar.dma_start(
                out=out_ext.rearrange("(i p) c -> p i c", i=2),
                in_=outs[:, :].rearrange("p (i c) -> p i c", i=2))

    nc.compile()
    return nc


def _get_nc():
    if "nc" not in _CACHE:
        _CACHE["nc"] = _build()
    return _CACHE["nc"]


def _prep_inputs(t, x, W1, b1, W2, b2, W3, b3):
    t = np.asarray(t, dtype=np.float32)
    x = np.ascontiguousarray(np.asarray(x, dtype=np.float32))
    W1 = np.asarray(W1, dtype=np.float32)
    b1 = np.asarray(b1, dtype=np.float32)
    w1s = np.concatenate([W1, b1.reshape(1, H)], axis=0)
    w1s = np.ascontiguousarray(
        w1s.reshape(D + 2, 16, 32).transpose(1, 0, 2))  # (16, 34, 32)
    W2 = np.ascontiguousarray(np.asarray(W2, dtype=np.float32))
    W3 = np.ascontiguousarray(np.asarray(W3, dtype=np.float32))
    colp = np.zeros((128, 11), dtype=np.float32)
    colp[:, 0] = 1.0
    colp[:, 1] = -1.0
    colp[:, 3:7] = np.asarray(b2, dtype=np.float32).reshape(NK, 128).T
    bias1 = (np.float32(t.ravel()[0]) * W1[D, :] + b1).astype(np.float32)
    colp[:, 7:11] = bias1.reshape(NK, 128).T
    rowv = np.ones(BC + D, dtype=np.float32)
    rowv[BC:] = np.asarray(b3, dtype=np.float32)
    rowp = np.zeros((16, 32), dtype=np.float32)
    rowp[:, 0:18] = rowv.reshape(16, 18)
    return x, w1s, W2, W3, colp, rowp


def kernel(t, x, W1, b1, W2, b2, W3, b3):
    from concourse.bass_utils import run_bass_kernel_spmd

    nc = _get_nc()
    x, w1s, W2, W3, colp, rowp = _prep_inputs(t, x, W1, b1, W2, b2, W3, b3)
    in_maps = []
    for i in range(NCORES):
        in_maps.append({
            "x": np.ascontiguousarray(x[i * BC:(i + 1) * BC]),
            "w1": w1s, "w2": W2, "w3": W3,
            "colp": colp, "rowp": rowp,
        })
    res = run_bass_kernel_spmd(nc, in_maps, core_ids=list(range(NCORES)))
    return np.concatenate([res.results[i]["out"] for i in range(NCORES)], axis=0)



# revision 4
# speedup vs baseline: 1.1523x; 1.1523x over previous
"""CNF vector-field + exact Jacobian-trace kernel for Trainium2 (8 NeuronCores).

Math: for each sample x (D=32), with inp = [x, t] (33,):
  h1 = tanh(inp @ W1 + b1); h2 = tanh(h1 @ W2 + b2); dx = h2 @ W3 + b3
  div = trace(J) = d1^T C d2,  C = W2 * (W3 @ W1r)^T  (elementwise),
  d_i = 1 - h_i^2,  W1r = W1[:32]
  out = [dx, div]  (B, 33)

v2 implementation notes (vs the transpose-heavy v1):
  - all layout work on HOST: x^T, W3^T, W3 chunk-packed, W2 row-chunks,
    biases folded into packed constant columns (bias1 = t*W1[32]+b1)
  - zero on-device transposes; output written feature-major (33, 256)
    and transposed on host after the gather
  - d1 = 1 - h1^2 computed directly (DVE square + two-scalar affine),
    which kills the vP row (2048 PE rows) and the ones-row bias matmuls
  - b3 applied via ACT-copy per-partition bias, not a matmul
  - 2 HWDGE rings: sync carries W2 as 4 chunk DMAs (z2 streams k-outer
    as chunks land); scalar carries big0 (x^T|W1|W3^T) + cpk consts
  - PE warmup spinner: N junk matmuls into a scratch PSUM ramp the PE
    DVFS clock during the DMA-wait window
  - z2 k-outer (stream W2), gt m-outer (early E/div tail pipelining)
"""
import sys

for _p in ("/opt/trn_rl_repo", "/root/.axon_site/_ro/trn_rl_repo"):
    if _p not in sys.path:
        sys.path.append(_p)

import numpy as np

B, D, H = 2048, 32, 512
NCORES = 8
BC = B // NCORES          # 256 rows per core
NK = H // 128             # 4 chunks of the hidden dim
WARMUP = 12               # PE clock-ramp spinner matmuls

_CACHE = {}


def _build():
    import concourse.bass as bass
    import concourse.tile as tile
    from concourse import bacc, mybir

    f32 = mybir.dt.float32
    f32r = mybir.dt.float32r
    AF = mybir.ActivationFunctionType
    ALU = mybir.AluOpType

    nc = bacc.Bacc("TRN2", target_bir_lowering=False, debug=False,
                   num_devices=NCORES)

    # big0 cols: [0:256]=x^T slice, [256:768]=W1r, [768:1280]=W3^T
    big0_ext = nc.dram_tensor("big0", [D, BC + 2 * H], f32r,
                              kind="ExternalInput").ap()
    w2_ext = nc.dram_tensor("w2", [H, H], f32r, kind="ExternalInput").ap()
    # cpk cols: [0:128]=W3 chunk-packed (lhsT for dx), 128=-1,
    # [129:133]=b2 col-major, [133:137]=bias1 col-major, 137=b3 (rows 0:32)
    cpk_ext = nc.dram_tensor("cpk", [128, 138], f32r,
                             kind="ExternalInput").ap()
    out_ext = nc.dram_tensor("out", [D + 1, BC], f32, kind="ExternalOutput").ap()

    with tile.TileContext(nc) as tc:
        with tc.tile_pool(name="const", bufs=1) as cpool, \
             tc.tile_pool(name="work", bufs=1) as wpool, \
             tc.tile_pool(name="ps", bufs=1, space="PSUM") as pps:

            def zps(nm):
                return pps.tile([128, BC], f32, name=nm, tag="z", bufs=4)

            def small_ps(nm, shape):
                return pps.tile(shape, f32, name=nm, tag="small", bufs=2)

            # -------- ACT spline-table preload (overlaps the DMA phase) ---
            dm0 = wpool.tile([1, 1], f32, name="dm0")
            dm1 = wpool.tile([1, 1], f32, name="dm1")
            nc.gpsimd.memset(dm0[:, :], 0.0)
            nc.scalar.activation(dm1[:, :], dm0[:, :], AF.Tanh)

            # -------- PE warmup spinner (ramps clock during DMA wait) -----
            wt = wpool.tile([1, BC], f32r, name="wt")
            nc.gpsimd.memset(wt[:, :].bitcast(f32), 0.0)
            warm = small_ps("warm", [1, BC])
            for _ in range(WARMUP):
                nc.tensor.matmul(warm[:, :], wt[:, 0:1], wt[:, :],
                                 start=True, stop=True)

            # ------------- input DMAs (two rings, need-ordered) -----------
            big0 = cpool.tile([D, BC + 2 * H], f32r, name="big0")
            nc.scalar.dma_start(out=big0[:, :], in_=big0_ext[:, :])
            xts = big0[:, 0:BC]
            w1p = big0[:, BC:BC + H]
            w3t = big0[:, BC + H:BC + 2 * H]

            w2all = cpool.tile([128, NK * H], f32r, name="w2all")
            for k in range(NK):
                nc.sync.dma_start(out=w2all[:, k * H:(k + 1) * H],
                                  in_=w2_ext[k * 128:(k + 1) * 128, :])
            w2k = [w2all[:, k * H:(k + 1) * H] for k in range(NK)]

            cpk = cpool.tile([128, 138], f32r, name="cpk")
            nc.scalar.dma_start(out=cpk[:, :], in_=cpk_ext[:, :])
            w3p = [cpk[:, k * D:(k + 1) * D] for k in range(NK)]
            neg_col = cpk[:, 128:129]

            # ---------------- layer 1 matmuls, then all tanh --------------
            z1s = []
            for m in range(NK):
                z1 = zps("z1")
                nc.tensor.matmul(z1[:, :], w1p[:, m * 128:(m + 1) * 128],
                                 xts[:, :], start=True, stop=True)
                z1s.append(z1)
            h1t = []
            for m in range(NK):
                h = wpool.tile([128, BC], f32r, name=f"h1t_{m}")
                nc.scalar.activation(h[:, :], z1s[m][:, :], AF.Tanh,
                                     bias=cpk[:, 133 + m:134 + m].bitcast(f32))
                h1t.append(h)

            # ---------------- C = W2 * M^T, M^T = W1r^T W3^T --------------
            cmat = cpool.tile([128, NK * H], f32r, name="cmat")
            for r in range(NK):
                mp = pps.tile([128, H], f32, name="mp", tag="mp", bufs=2)
                nc.tensor.matmul(mp[:, :], w1p[:, r * 128:(r + 1) * 128],
                                 w3t[:, :], start=True, stop=True)
                nc.vector.tensor_tensor(out=cmat[:, r * H:(r + 1) * H],
                                        in0=w2k[r].bitcast(f32),
                                        in1=mp[:, :], op=ALU.mult)
            cmk = [cmat[:, k * H:(k + 1) * H] for k in range(NK)]

            # ---------------- d1 = 1 - h1^2 on DVE ------------------------
            d1t = []
            for m in range(NK):
                sq = wpool.tile([128, BC], f32, name=f"sq_{m}")
                nc.vector.tensor_tensor(out=sq[:, :], in0=h1t[m][:, :].bitcast(f32),
                                        in1=h1t[m][:, :].bitcast(f32), op=ALU.mult)
                d1 = wpool.tile([128, BC], f32r, name=f"d1_{m}")
                nc.vector.tensor_scalar(out=d1[:, :], in0=sq[:, :],
                                        scalar1=-1.0, scalar2=1.0,
                                        op0=ALU.mult, op1=ALU.add)
                d1t.append(d1)

            # ---------------- layer 2: k-outer streams W2 chunks ----------
            z2s = [zps("z2") for _ in range(NK)]
            for k in range(NK):
                for m in range(NK):
                    nc.tensor.matmul(z2s[m][:, :],
                                     w2k[k][:, m * 128:(m + 1) * 128],
                                     h1t[k][:, :],
                                     start=(k == 0), stop=(k == NK - 1))
            h2t = []
            for m in range(NK):
                h = wpool.tile([128, BC], f32r, name=f"h2t_{m}")
                nc.scalar.activation(h[:, :], z2s[m][:, :], AF.Tanh,
                                     bias=cpk[:, 129 + m:130 + m].bitcast(f32))
                h2t.append(h)

            # ---------------- h2sq on GpSimd (SBUF only) ------------------
            h2sq = []
            for m in range(NK):
                s2 = wpool.tile([128, BC], f32, name=f"h2sq_{m}")
                nc.gpsimd.tensor_tensor(out=s2[:, :], in0=h2t[m][:, :].bitcast(f32),
                                        in1=h2t[m][:, :].bitcast(f32), op=ALU.mult)
                h2sq.append(s2)

            # ------- gt = C^T d1 (m-outer: early E/div tail) --------------
            # E = (h2sq - 1) * gt = -d2*gt ; div = (-1)^T sum E
            ee = []
            for m in range(NK):
                gt = zps("gt")
                for k in range(NK):
                    nc.tensor.matmul(gt[:, :],
                                     cmk[k][:, m * 128:(m + 1) * 128],
                                     d1t[k][:, :],
                                     start=(k == 0), stop=(k == NK - 1))
                e = wpool.tile([128, BC], f32r, name=f"e_{m}")
                nc.vector.scalar_tensor_tensor(out=e[:, :], in0=h2sq[m][:, :],
                                               scalar=1.0, in1=gt[:, :],
                                               op0=ALU.subtract, op1=ALU.mult)
                ee.append(e)

            # -------- dx = W3^T h2 (+b3 via copy-bias); div = (-1)^T E ----
            outt = wpool.tile([D + 1, BC], f32, name="outt")
            dx_ps = small_ps("dx_ps", [D, BC])
            for k in range(NK):
                nc.tensor.matmul(dx_ps[:, :], w3p[k], h2t[k][:, :],
                                 start=(k == 0), stop=(k == NK - 1))
            nc.scalar.activation(outt[0:D, :], dx_ps[:, :], AF.Identity,
                                 bias=cpk[0:D, 137:138].bitcast(f32))
            div_ps = small_ps("div_ps", [1, BC])
            for k in range(NK):
                nc.tensor.matmul(div_ps[:, :], neg_col, ee[k][:, :],
                                 start=(k == 0), stop=(k == NK - 1))
            nc.scalar.activation(outt[D:D + 1, :], div_ps[:, :], AF.Copy)

            # ------- store feature-major; host transposes -----------------
            nc.scalar.dma_start(out=out_ext[:, :], in_=outt[:, :])

    nc.compile()
    return nc


def _get_nc():
    if "nc" not in _CACHE:
        _CACHE["nc"] = _build()
    return _CACHE["nc"]


def _prep_in_maps(t, x, W1, b1, W2, b2, W3, b3):
    t = np.asarray(t, dtype=np.float32)
    x = np.asarray(x, dtype=np.float32)
    W1 = np.asarray(W1, dtype=np.float32)
    b1 = np.asarray(b1, dtype=np.float32)
    W2 = np.ascontiguousarray(np.asarray(W2, dtype=np.float32))
    W3 = np.asarray(W3, dtype=np.float32)

    xT = x[:, :D].T                                    # (32, 2048)
    w1p = W1[:D]                                       # (32, 512)
    w3t = W3.T                                         # (32, 512)

    cpk = np.zeros((128, 138), dtype=np.float32)
    cpk[:, 0:128] = W3.reshape(NK, 128, D).transpose(1, 0, 2).reshape(128, 128)
    cpk[:, 128] = -1.0
    cpk[:, 129:133] = np.asarray(b2, dtype=np.float32).reshape(NK, 128).T
    bias1 = (np.float32(t.ravel()[0]) * W1[D, :] + b1).astype(np.float32)
    cpk[:, 133:137] = bias1.reshape(NK, 128).T
    cpk[0:D, 137] = np.asarray(b3, dtype=np.float32)

    shared = np.concatenate([w1p, w3t], axis=1)        # (32, 1024)
    in_maps = []
    for i in range(NCORES):
        big0 = np.concatenate([xT[:, i * BC:(i + 1) * BC], shared], axis=1)
        in_maps.append({
            "big0": np.ascontiguousarray(big0),
            "w2": W2, "cpk": cpk,
        })
    return in_maps


def kernel(t, x, W1, b1, W2, b2, W3, b3):
    from concourse.bass_utils import run_bass_kernel_spmd

    nc = _get_nc()
    in_maps = _prep_in_maps(t, x, W1, b1, W2, b2, W3, b3)
    res = run_bass_kernel_spmd(nc, in_maps, core_ids=list(range(NCORES)))
    return np.concatenate(
        [np.ascontiguousarray(res.results[i]["out"].T) for i in range(NCORES)],
        axis=0)


# revision 10
# speedup vs baseline: 1.2908x; 1.1202x over previous
"""CNF vector-field + exact Jacobian-trace kernel for Trainium2 (8 NeuronCores).

Math: for each sample x (D=32), with inp = [x, t] (33,):
  h1 = tanh(inp @ W1 + b1); h2 = tanh(h1 @ W2 + b2); dx = h2 @ W3 + b3
  div = trace(J) = d1^T C d2,  C = W2 * (W3 @ W1r)^T  (elementwise),
  d_i = 1 - h_i^2,  W1r = W1[:32]
  out = [dx, div]  (B, 33)

v3 implementation notes:
  - all layout work on HOST: x^T, W3^T, W3 chunk-packed, biases folded
    into packed constant columns (bias1 = t*W1[32]+b1)
  - full bf16 datapath (PSUM accumulate stays f32): halves HBM wire to
    ~0.6MB and doubles DVE/GpSimd element throughput; measured rel err
    ~5e-3 vs the 2e-2 gate
  - d1 = 1 - h1^2 computed directly (GpSimd square + DVE affine), no
    vP row / ones-row matmuls; b3 via ACT Identity copy-bias
  - PSUM half-tile packing: [128,512] banks hold two [128,256]
    accumulators, so z1/z2/gt cycle through 4 banks without waiting on
    the tanh2 chain
  - two HWDGE rings: sync carries hot consts + W2 as 4 chunk DMAs
    (z2 streams k-outer as chunks land); scalar carries big0 + w3pn;
    ACT table preload emitted after the scalar-ring issues
  - PE warmup spinner ramps the DVFS clock during the DMA-wait window
"""
import sys

for _p in ("/opt/trn_rl_repo", "/root/.axon_site/_ro/trn_rl_repo"):
    if _p not in sys.path:
        sys.path.append(_p)

import numpy as np
import ml_dtypes

BF16 = ml_dtypes.bfloat16
B, D, H = 2048, 32, 512
NCORES = 8
BC = B // NCORES          # 256 rows per core
NK = H // 128             # 4 chunks of the hidden dim
WARMUP = 12               # PE clock-ramp spinner matmuls

_CACHE = {}


def _build():
    import concourse.bass as bass
    import concourse.tile as tile
    from concourse import bacc, mybir

    f32 = mybir.dt.float32
    bf = mybir.dt.bfloat16
    AF = mybir.ActivationFunctionType
    ALU = mybir.AluOpType

    nc = bacc.Bacc("TRN2", target_bir_lowering=False, debug=False,
                   num_devices=NCORES)

    # big0 cols: [0:256]=x^T slice, [256:768]=W1r, [768:1280]=W3^T
    big0_ext = nc.dram_tensor("big0", [D, BC + 2 * H], bf,
                              kind="ExternalInput").ap()
    w2_ext = nc.dram_tensor("w2", [H, H], bf, kind="ExternalInput").ap()
    # w3pn cols: [0:128]=W3 chunk-packed (lhsT for dx), 128=-1
    w3pn_ext = nc.dram_tensor("w3pn", [128, 129], bf,
                              kind="ExternalInput").ap()
    # hot cols: [0:4]=b2 col-major, [4:8]=bias1 col-major, 8=b3 (rows 0:32)
    hot_ext = nc.dram_tensor("hot", [128, 9], f32, kind="ExternalInput").ap()
    out_ext = nc.dram_tensor("out", [D + 1, BC], f32, kind="ExternalOutput").ap()

    with tile.TileContext(nc) as tc:
        with tc.tile_pool(name="const", bufs=1) as cpool, \
             tc.tile_pool(name="work", bufs=1) as wpool, \
             tc.tile_pool(name="ps", bufs=1, space="PSUM") as pps:

            # NOTE: interleaving open accumulation groups within one PSUM
            # bank corrupts the PE (hw-verified) — every accumulator below
            # owns a full bank while open.
            def zps(nm):
                return pps.tile([128, BC], f32, name=nm, tag="z", bufs=4)

            def mps(nm):
                return pps.tile([128, H], f32, name=nm, tag="mp", bufs=2)

            def small_ps(nm, shape):
                return pps.tile(shape, f32, name=nm, tag="small", bufs=2)

            # -------- PE warmup spinner (ramps clock during DMA wait) -----
            wt = wpool.tile([1, BC], bf, name="wt")
            nc.gpsimd.memset(wt[:, :], 0.0)
            warm = small_ps("warm", [1, BC])
            for _ in range(WARMUP):
                nc.tensor.matmul(warm[:, :], wt[:, 0:1], wt[:, :],
                                 start=True, stop=True)

            # ------------- input DMAs (two rings, need-ordered) -----------
            hot = cpool.tile([128, 9], f32, name="hot")
            nc.sync.dma_start(out=hot[:, :], in_=hot_ext[:, :])

            big0 = cpool.tile([D, BC + 2 * H], bf, name="big0")
            nc.scalar.dma_start(out=big0[:, :], in_=big0_ext[:, :])
            xts = big0[:, 0:BC]
            w1p = big0[:, BC:BC + H]
            w3t = big0[:, BC + H:BC + 2 * H]

            w2all = cpool.tile([128, NK * H], bf, name="w2all")
            for k in range(NK):
                nc.sync.dma_start(out=w2all[:, k * H:(k + 1) * H],
                                  in_=w2_ext[k * 128:(k + 1) * 128, :])
            w2k = [w2all[:, k * H:(k + 1) * H] for k in range(NK)]

            w3pn = cpool.tile([128, 129], bf, name="w3pn")
            nc.scalar.dma_start(out=w3pn[:, :], in_=w3pn_ext[:, :])
            w3p = [w3pn[:, k * D:(k + 1) * D] for k in range(NK)]
            neg_col = w3pn[:, 128:129]

            # -------- ACT spline-table preload (after scalar issues) ------
            dm0 = wpool.tile([1, 1], f32, name="dm0")
            dm1 = wpool.tile([1, 1], f32, name="dm1")
            nc.gpsimd.memset(dm0[:, :], 0.0)
            nc.scalar.activation(dm1[:, :], dm0[:, :], AF.Tanh)

            # ---------------- layer 1 matmuls, then all tanh --------------
            z1s = []
            for m in range(NK):
                z1 = zps("z1")
                nc.tensor.matmul(z1[:, :], w1p[:, m * 128:(m + 1) * 128],
                                 xts[:, :], start=True, stop=True)
                z1s.append(z1)
            h1t = []
            for m in range(NK):
                h = wpool.tile([128, BC], bf, name=f"h1t_{m}")
                nc.scalar.activation(h[:, :], z1s[m][:, :], AF.Tanh,
                                     bias=hot[:, 4 + m:5 + m])
                h1t.append(h)

            # ---- C = W2 * M^T (M^T = W1r^T W3^T) interleaved with z2 -----
            cmat = cpool.tile([128, NK * H], bf, name="cmat")
            cmk = [cmat[:, k * H:(k + 1) * H] for k in range(NK)]
            z2s = [zps("z2") for _ in range(NK)]

            def emit_mp(r):
                mp = mps("mp")
                nc.tensor.matmul(mp[:, :], w1p[:, r * 128:(r + 1) * 128],
                                 w3t[:, :], start=True, stop=True)
                nc.vector.tensor_tensor(out=cmk[r], in0=w2k[r],
                                        in1=mp[:, :], op=ALU.mult)

            def emit_z2_round(k):
                for m in range(NK):
                    nc.tensor.matmul(z2s[m][:, :],
                                     w2k[k][:, m * 128:(m + 1) * 128],
                                     h1t[k][:, :],
                                     start=(k == 0), stop=(k == NK - 1))

            emit_mp(0)
            emit_z2_round(0)
            emit_mp(1)
            emit_z2_round(1)
            emit_mp(2)
            emit_z2_round(2)
            emit_mp(3)
            emit_z2_round(3)

            # ---------------- d1 = 1 - h1^2 (GpSimd sq, DVE affine) -------
            d1t = []
            for m in range(NK):
                sq = wpool.tile([128, BC], bf, name=f"sq_{m}")
                nc.gpsimd.tensor_tensor(out=sq[:, :], in0=h1t[m][:, :],
                                        in1=h1t[m][:, :], op=ALU.mult)
                d1 = wpool.tile([128, BC], bf, name=f"d1_{m}")
                nc.vector.tensor_scalar(out=d1[:, :], in0=sq[:, :],
                                        scalar1=-1.0, scalar2=1.0,
                                        op0=ALU.mult, op1=ALU.add)
                d1t.append(d1)

            h2t = []
            for m in range(NK):
                h = wpool.tile([128, BC], bf, name=f"h2t_{m}")
                nc.scalar.activation(h[:, :], z2s[m][:, :], AF.Tanh,
                                     bias=hot[:, m:1 + m])
                h2t.append(h)

            # ---------------- h2sq on GpSimd ------------------------------
            h2sq = []
            for m in range(NK):
                s2 = wpool.tile([128, BC], bf, name=f"h2sq_{m}")
                nc.gpsimd.tensor_tensor(out=s2[:, :], in0=h2t[m][:, :],
                                        in1=h2t[m][:, :], op=ALU.mult)
                h2sq.append(s2)

            # ------- gt = C^T d1 (m-outer, cycles 2 mp-tag banks) ---------
            # E = (h2sq - 1) * gt = -d2*gt ; div = (-1)^T sum E
            ee = []
            for m in range(NK):
                gt = mps("gt")
                for k in range(NK):
                    nc.tensor.matmul(gt[:, 0:BC],
                                     cmk[k][:, m * 128:(m + 1) * 128],
                                     d1t[k][:, :],
                                     start=(k == 0), stop=(k == NK - 1))
                e = wpool.tile([128, BC], bf, name=f"e_{m}")
                nc.vector.scalar_tensor_tensor(out=e[:, :], in0=h2sq[m][:, :],
                                               scalar=1.0, in1=gt[:, 0:BC],
                                               op0=ALU.subtract, op1=ALU.mult)
                ee.append(e)

            # -------- dx = W3^T h2 (+b3 via copy-bias) --------------------
            outt = wpool.tile([D + 1, BC], f32, name="outt")
            dx_ps = small_ps("dx_ps", [D, BC])
            for k in range(NK):
                nc.tensor.matmul(dx_ps[:, :], w3p[k], h2t[k][:, :],
                                 start=(k == 0), stop=(k == NK - 1))
            nc.scalar.activation(outt[0:D, :], dx_ps[:, :], AF.Identity,
                                 bias=hot[0:D, 8:9])
            div_ps = small_ps("div_ps", [1, BC])
            for k in range(NK):
                nc.tensor.matmul(div_ps[:, :], neg_col, ee[k][:, :],
                                 start=(k == 0), stop=(k == NK - 1))
            nc.scalar.activation(outt[D:D + 1, :], div_ps[:, :], AF.Copy)

            # ------- store feature-major; host transposes -----------------
            nc.sync.dma_start(out=out_ext[:, :], in_=outt[:, :])

    nc.compile()
    return nc


def _get_nc():
    if "nc" not in _CACHE:
        _CACHE["nc"] = _build()
    return _CACHE["nc"]


def _prep_in_maps(t, x, W1, b1, W2, b2, W3, b3):
    t = np.asarray(t, dtype=np.float32)
    x = np.asarray(x, dtype=np.float32)
    W1 = np.asarray(W1, dtype=np.float32)
    b1 = np.asarray(b1, dtype=np.float32)
    W2 = np.asarray(W2, dtype=np.float32)
    W3 = np.asarray(W3, dtype=np.float32)

    xT = x[:, :D].T.astype(BF16)                       # (32, 2048)
    w1p = W1[:D].astype(BF16)                          # (32, 512)
    w3t = W3.T.astype(BF16)                            # (32, 512)
    w2b = np.ascontiguousarray(W2.astype(BF16))

    w3pn = np.zeros((128, 129), dtype=BF16)
    w3pn[:, 0:128] = W3.reshape(NK, 128, D).transpose(1, 0, 2).reshape(128, 128).astype(BF16)
    w3pn[:, 128] = BF16(-1.0)

    hot = np.zeros((128, 9), dtype=np.float32)
    hot[:, 0:4] = np.asarray(b2, dtype=np.float32).reshape(NK, 128).T
    bias1 = (np.float32(t.ravel()[0]) * W1[D, :] + b1).astype(np.float32)
    hot[:, 4:8] = bias1.reshape(NK, 128).T
    hot[0:D, 8] = np.asarray(b3, dtype=np.float32)

    shared = np.concatenate([w1p, w3t], axis=1)        # (32, 1024) bf16
    in_maps = []
    for i in range(NCORES):
        big0 = np.concatenate([xT[:, i * BC:(i + 1) * BC], shared], axis=1)
        in_maps.append({
            "big0": np.ascontiguousarray(big0),
            "w2": w2b, "w3pn": w3pn, "hot": hot,
        })
    return in_maps


def kernel(t, x, W1, b1, W2, b2, W3, b3):
    from concourse.bass_utils import run_bass_kernel_spmd

    nc = _get_nc()
    in_maps = _prep_in_maps(t, x, W1, b1, W2, b2, W3, b3)
    res = run_bass_kernel_spmd(nc, in_maps, core_ids=list(range(NCORES)))
    return np.concatenate(
        [np.ascontiguousarray(res.results[i]["out"].T) for i in range(NCORES)],
        axis=0)
